# revision 3
# baseline (speedup 1.0000x reference)
"""GQA attention (16 Q heads / 4 KV heads, head_dim 128, RoPE, varlen causal)
on 8 Trainium2 NeuronCores, tensor-parallel over heads.

Per core c: Q heads {2c, 2c+1}, KV head c//2.
Pipeline per 1024-token group: QKV projection (bf16, th-paired PSUM banks,
d-outer loops so consecutive matmuls share the stationary operand) -> RoPE
(permutation-matmul + DVE, th-paired) -> block-sparse S^T attention:
key-block PAIRS share a 2-bank PSUM tile; varlen-causal masking is applied
by triangle-bias matmuls accumulated into S (exp -> 0); one exp ACT per
pair; GpSimd fp8 cast; softmax denominators via fp8 DoubleRow ones-matmul
(two key blocks per pass); PV in bf16 -> normalize -> chunked AllToAll
(each core ends with the full attention output for its 512-token slice)
-> output projection (g-outer, full wo resident) on that slice.

DMAs are batched (one descriptor-rich dma_start per big tile) to keep the
SP sequencer off the critical path.
"""
import os
import sys

for _p in ("/opt/trn_rl_repo",):
    if _p not in sys.path:
        sys.path.insert(0, _p)

import numpy as np
import ml_dtypes

import concourse.bass as bass
import concourse.tile as tile
from concourse import bacc, mybir
from concourse.bass_utils import run_bass_kernel_spmd
from concourse.masks import make_identity

BF16 = ml_dtypes.bfloat16
DT = mybir.dt.bfloat16
F32 = mybir.dt.float32
FP8 = mybir.dt.float8e4

T, DIM, HEADS, KVH, HD = 4096, 2048, 16, 4, 128
NCORES = 8
QH = HEADS // NCORES            # q heads per core = 2
WCOLS = QH * HD + 2 * HD        # wqkv cols per core = 512
TT = 512                        # query tile (psum bank free dim)
NTT = T // TT                   # 8
NSB = T // 128                  # 32 key blocks
TG = 1024                       # phase-1 token group
NTG = T // TG                   # 4
DBLK = DIM // 128               # 16 contraction blocks
NEGB = -30000.0                 # bias value: exp(x + NEGB) == 0
DEN_BF16 = bool(os.environ.get("DEN_BF16"))
DEN_FP8PLAIN = bool(os.environ.get("DEN_FP8PLAIN"))
CAST_DVE = os.environ.get("CAST_DVE", "1") == "1"
FORCE_MASKS = bool(os.environ.get("FORCE_MASKS"))


class Blk:
    __slots__ = ("sb", "lo", "tri", "mi")

    def __init__(self, sb, lo, tri, mi):
        self.sb, self.lo, self.tri, self.mi = sb, lo, tri, mi


def _block_structure(seq_ids):
    """Per query-tile list of allowed 128-key blocks.

    fast=True when every partial block is a causal triangle (allowed iff
    col >= row + (s0 - t0)); then no mask tensors are needed.
    """
    seg = np.asarray(seq_ids).astype(np.int64)
    idx = np.arange(T)
    allowed = (seg[:, None] == seg[None, :]) & (idx[:, None] <= idx[None, :])
    block_list, masks = [], []
    fast = True
    for tt in range(NTT):
        t0 = tt * TT
        lst = []
        for sb in range(NSB):
            s0 = sb * 128
            blk = allowed[s0:s0 + 128, t0:t0 + TT]
            if not blk.any():
                continue
            if blk.all():
                lst.append(Blk(sb, 0, False, None))
                continue
            lo = s0 - t0
            if 0 <= lo < TT:
                cols = np.arange(TT)[None, :]
                rows = np.arange(128)[:, None]
                if np.array_equal(blk, cols >= rows + lo):
                    lst.append(Blk(sb, lo, True, None))
                    continue
            fast = False
            live = np.flatnonzero(blk.any(axis=0))
            masks.append(blk)
            lst.append(Blk(sb, int(live[0]), False, len(masks) - 1))
        block_list.append(lst)
    if masks:
        masks_arr = np.stack(masks).astype(BF16)
    else:
        masks_arr = np.zeros((1, 128, TT), BF16)
    return block_list, fast, masks_arr


def _pairs_for_tile(blocks):
    """Group blocks into 2-bank pairs; first pair must span [0, TT)."""
    zeros = [b for b in blocks if b.lo == 0]
    rest = sorted([b for b in blocks if b.lo > 0], key=lambda b: b.lo)
    assert zeros, "every query tile needs a lo=0 block"
    pairs = []
    if len(zeros) >= 2:
        pairs.append((zeros[0], zeros[1]))
        pool = zeros[2:] + rest
    else:
        if rest:
            pairs.append((zeros[0], rest.pop(0)))
        else:
            pairs.append((zeros[0], None))
        pool = rest
    for i in range(0, len(pool) - 1, 2):
        pairs.append((pool[i], pool[i + 1]))
    if len(pool) % 2:
        pairs.append((pool[-1], None))
    return pairs


def _build_program(block_list, fast, n_masks):
    nc = bacc.Bacc("TRN2", target_bir_lowering=False, debug=False,
                   num_devices=NCORES)
    xT_d = nc.dram_tensor("xT", [DIM, T], DT, kind="ExternalInput")
    wqkv_d = nc.dram_tensor("wqkv", [DIM, WCOLS], DT, kind="ExternalInput")
    wo_d = nc.dram_tensor("wo", [DIM, DIM], DT, kind="ExternalInput")
    cos2_d = nc.dram_tensor("cos2", [HD, T], DT, kind="ExternalInput")
    sin2_d = nc.dram_tensor("sin2", [HD, T], DT, kind="ExternalInput")
    p64_d = nc.dram_tensor("p64", [HD, HD], DT, kind="ExternalInput")
    lb_d = nc.dram_tensor("lb", [128, 128], DT, kind="ExternalInput")
    masks_d = nc.dram_tensor("masks", [n_masks, 128, TT], DT,
                             kind="ExternalInput")
    out_d = nc.dram_tensor("out", [TT, DIM], F32, kind="ExternalOutput")

    EXP = mybir.ActivationFunctionType.Exp
    COPY = mybir.ActivationFunctionType.Copy
    DR = mybir.MatmulPerfMode.DoubleRow

    xT_r = xT_d[:].rearrange("(d p) t -> p d t", p=128)
    wqkv_r = wqkv_d[:].rearrange("(d p) c -> p d c", p=128)
    wo_r = wo_d[:].rearrange("(jb p) c -> p jb c", p=128)

    pair_list = [_pairs_for_tile(bl) for bl in block_list]

    with tile.TileContext(nc) as tc:
        with tc.tile_pool(name="persist", bufs=1) as persist, \
             tc.tile_pool(name="p3early", bufs=1) as p3e, \
             tc.tile_pool(name="dram", bufs=1, space="DRAM") as dram:
            QT0 = persist.tile([HD, T], DT, name="QT0")
            QT1 = persist.tile([HD, T], DT, name="QT1")
            QT = [QT0, QT1]
            KT = persist.tile([HD, T], DT, name="KT")
            Vn = persist.tile([HD, T], DT, name="Vn")
            ones_bf = persist.tile([128, 128], DT, name="ones_bf")
            nc.vector.memset(ones_bf[:], 1.0)
            fours_bf = persist.tile([128, 128], DT, name="fours_bf")
            nc.vector.memset(fours_bf[:], 4.0)
            ones8 = persist.tile([128, 2, 128], FP8, name="ones8")
            nc.vector.tensor_copy(ones8[:, 0, :], fours_bf[:])
            nc.vector.tensor_copy(ones8[:, 1, :], fours_bf[:])
            ident = persist.tile([128, 128], DT, name="ident")
            make_identity(nc, ident[:])
            negb = persist.tile([128, 128], DT, name="negb")
            nc.vector.memset(negb[:], NEGB)
            lb_sb = persist.tile([128, 128], DT, name="lb_sb")
            nc.scalar.dma_start(out=lb_sb[:], in_=lb_d[:])
            p64_sb = persist.tile([HD, HD], DT, name="p64_sb")
            nc.scalar.dma_start(out=p64_sb[:], in_=p64_d[:])
            # weights as lhsT tiles: w_sb[p, d, j]; d=0 shipped alone so the
            # first matmul chain can start early
            w_sb = persist.tile([128, DBLK, WCOLS], DT, name="w_sb")
            nc.scalar.dma_start(out=w_sb[:, 0, :], in_=wqkv_r[:, 0, :])
            for dlo, dhi in ((1, 4), (4, 8), (8, 12), (12, DBLK)):
                nc.scalar.dma_start(out=w_sb[:, dlo:dhi, :],
                                    in_=wqkv_r[:, dlo:dhi, :])
            wo_res0 = persist.tile([128, DBLK, DIM // 2], DT,
                                   name="wo_res0")
            for jlo, jhi in ((0, 8), (8, DBLK)):
                nc.scalar.dma_start(
                    out=wo_res0[:, jlo:jhi, :],
                    in_=wo_r[:, jlo:jhi, 0:DIM // 2])
            masks_sb = None
            if not fast:
                masks_sb = persist.tile([128, n_masks, TT], DT,
                                        name="masks_sb")
                nc.gpsimd.dma_start(
                    out=masks_sb[:],
                    in_=masks_d[:].rearrange("m p t -> p m t"))

            # chunked all-to-all staging (outs in Shared DRAM for fast CC)
            attT_perm = [dram.tile([NCORES, QH * HD, 128], DT,
                                   name=f"attT_perm{g}") for g in range(NTG)]
            a2a_out = [dram.tile([DIM, 128], DT, name=f"a2a_out{g}")
                       for g in range(NTG)]

            # ---------------- phase 1 + 2: projection, rope, attention ----
            with tc.tile_pool(name="p1sbuf", bufs=1) as p1s, \
                 tc.tile_pool(name="xpool", bufs=3) as xpool, \
                 tc.tile_pool(name="p1tmp", bufs=2) as p1t, \
                 tc.tile_pool(name="prpsum", bufs=1, space="PSUM") as prp, \
                 tc.tile_pool(name="atpsum", bufs=1, space="PSUM") as atp, \
                 tc.tile_pool(name="atsbuf", bufs=1) as ats:
                cos_sb = p1s.tile([HD, T], DT, name="cos_sb")
                sin_sb = p1s.tile([HD, T], DT, name="sin_sb")
                rope_loaded = [False]

                def load_rope_tables():
                    if not rope_loaded[0]:
                        rope_loaded[0] = True
                        nc.gpsimd.dma_start(out=cos_sb[:], in_=cos2_d[:])
                        nc.gpsimd.dma_start(out=sin_sb[:], in_=sin2_d[:])

                def pair_psum():
                    return prp.tile([128, 2, TT], F32, name="pp", tag="pp",
                                    bufs=2)

                # wqkv col groups: q0@0, q1@HD, k@2HD, v@3HD; process k,v
                # first so attention deps resolve early
                JSLICE = {"q0": 0, "q1": HD, "k": QH * HD, "v": QH * HD + HD}
                attS = [None] * NTG
                for tg in range(NTG):
                    g0 = tg * TG
                    xt = []
                    for th in range(TG // TT):
                        xh = xpool.tile([128, DBLK, TT], DT, name="xtile",
                                        tag="xtile", bufs=3)
                        c0 = g0 + th * TT
                        if tg == 0:
                            for q in range(4):
                                nc.sync.dma_start(
                                    out=xh[:, 4 * q:4 * (q + 1), :],
                                    in_=xT_r[:, 4 * q:4 * (q + 1),
                                             c0:c0 + TT])
                        else:
                            nc.sync.dma_start(out=xh[:],
                                              in_=xT_r[:, :, c0:c0 + TT])
                        xt.append(xh)
                    load_rope_tables()
                    for jname in ("k", "v", "q0", "q1"):
                        j0 = JSLICE[jname]
                        pp = pair_psum()
                        for d in range(DBLK):
                            for th in range(TG // TT):
                                nc.tensor.matmul(
                                    pp[:, th, :],
                                    lhsT=w_sb[:, d, j0:j0 + HD],
                                    rhs=xt[th][:, d, :],
                                    start=(d == 0), stop=(d == DBLK - 1),
                                    skip_group_check=True)
                        if jname == "v":
                            vt = p1t.tile([128, 2, TT], DT, name="vt",
                                          bufs=2)
                            nc.vector.tensor_copy(vt[:], pp[:])
                            for th in range(TG // TT):
                                vtr = prp.tile([128, 4, 128], DT,
                                               name="vtr", tag="ptmp",
                                               bufs=1,
                                               padded_shape=[128, 4, 256])
                                for i in range(TT // 128):
                                    nc.tensor.transpose(
                                        vtr[:, i, :],
                                        vt[:, th, i * 128:(i + 1) * 128],
                                        ident[:])
                                s0 = g0 + th * TT
                                nc.vector.tensor_copy(
                                    Vn[:, s0:s0 + TT].rearrange(
                                        "p (a b) -> p a b", a=4),
                                    vtr[:])
                        else:
                            dst = {"k": KT, "q0": QT0, "q1": QT1}[jname]
                            raw = p1t.tile([128, 2, TT], DT, name="raw",
                                           bufs=2)
                            nc.vector.tensor_copy(raw[:], pp[:])
                            t1 = p1t.tile([128, 2, TT], DT, name="t1",
                                          bufs=2)
                            nc.vector.tensor_mul(
                                t1[:].rearrange("p a b -> p (a b)"),
                                raw[:].rearrange("p a b -> p (a b)"),
                                cos_sb[:, g0:g0 + TG])
                            t2 = p1t.tile([128, 2, TT], DT, name="t2",
                                          bufs=2)
                            for th in range(TG // TT):
                                psw = prp.tile([128, TT], F32, name="psw",
                                               tag="ptmp", bufs=1)
                                nc.tensor.matmul(psw[:], lhsT=p64_sb[:],
                                                 rhs=raw[:, th, :],
                                                 start=True, stop=True)
                                c0 = g0 + th * TT
                                nc.vector.tensor_mul(
                                    t2[:, th, :], psw[:],
                                    sin_sb[:, c0:c0 + TT])
                            nc.vector.tensor_add(
                                dst[:, g0:g0 + TG],
                                t1[:].rearrange("p a b -> p (a b)"),
                                t2[:].rearrange("p a b -> p (a b)"))

                    # -------- attention for this group's query tiles ------
                    for tt in (2 * tg, 2 * tg + 1):
                        for h in range(QH):
                            t0 = tt * TT
                            pairs = pair_list[tt]
                            npair = len(pairs)
                            fp8_ok = len(block_list[tt]) >= 8
                            pOT = atp.tile([128, TT], F32, name="pOT",
                                           bufs=2)
                            pSUM = atp.tile([128, TT], F32, name="pSUM",
                                            bufs=1)
                            for pi, (b0, b1) in enumerate(pairs):
                                blks = [b0] if b1 is None else [b0, b1]
                                pS = pair_psum()
                                pair_lo = min(b.lo for b in blks)
                                for bi, b in enumerate(blks):
                                    s0 = b.sb * 128
                                    # ops hitting this bank: S, [triangle],
                                    # [fill]; stop on the last one
                                    n_ops = 1 + (1 if b.tri else 0) \
                                        + ((b.lo - pair_lo + 127) // 128
                                           if b.lo > pair_lo else 0)
                                    k = 0
                                    nc.tensor.matmul(
                                        pS[:, bi, b.lo:TT],
                                        lhsT=KT[:, s0:s0 + 128],
                                        rhs=QT[h][:, t0 + b.lo:t0 + TT],
                                        start=True, stop=(n_ops == 1),
                                        skip_group_check=True)
                                    k += 1
                                    if b.tri:
                                        k += 1
                                        nc.tensor.matmul(
                                            pS[:, bi, b.lo:b.lo + 128],
                                            lhsT=lb_sb[:], rhs=ident[:],
                                            start=False, stop=(k == n_ops),
                                            skip_group_check=True)
                                    f0 = pair_lo
                                    while f0 < b.lo:
                                        w = min(128, b.lo - f0)
                                        k += 1
                                        nc.tensor.matmul(
                                            pS[:, bi, f0:f0 + w],
                                            lhsT=negb[:],
                                            rhs=ident[:, 0:w],
                                            start=True, stop=(k == n_ops),
                                            skip_group_check=True)
                                        f0 += w
                                expP = ats.tile([128, 2, TT], DT,
                                                name="expP", tag="expP",
                                                bufs=4)
                                if b1 is None:
                                    nc.scalar.activation(
                                        expP[:, 0, pair_lo:TT],
                                        pS[:, 0, pair_lo:TT], EXP)
                                else:
                                    nc.scalar.activation(
                                        expP[:, :, pair_lo:TT],
                                        pS[:, :, pair_lo:TT], EXP)
                                if not fast:
                                    for bi, b in enumerate(blks):
                                        if b.mi is not None:
                                            nc.vector.tensor_mul(
                                                expP[:, bi, b.lo:TT],
                                                expP[:, bi, b.lo:TT],
                                                masks_sb[:, b.mi, b.lo:TT])
                                # softmax denominators
                                if (b1 is not None and not DEN_BF16
                                        and fp8_ok):
                                    exp8 = ats.tile([128, 2, TT], FP8,
                                                    name="exp8", tag="exp8",
                                                    bufs=3)
                                    (nc.vector if CAST_DVE else nc.gpsimd
                                     ).tensor_scalar_mul(
                                        exp8[:, :, pair_lo:TT],
                                        expP[:, :, pair_lo:TT], 0.25)
                                    if DEN_FP8PLAIN:
                                        for bi2 in range(2):
                                            nc.tensor.matmul(
                                                pSUM[:, pair_lo:TT],
                                                lhsT=ones8[:, 0, :],
                                                rhs=exp8[:, bi2, pair_lo:TT],
                                                start=(pi == 0 and bi2 == 0),
                                                stop=(pi == npair - 1
                                                      and bi2 == 1),
                                                skip_group_check=True)
                                    else:
                                        nc.tensor.matmul(
                                            pSUM[:, pair_lo:TT],
                                            lhsT=ones8[:],
                                            rhs=exp8[:, :, pair_lo:TT],
                                            start=(pi == 0),
                                            stop=(pi == npair - 1),
                                            perf_mode=DR,
                                            skip_group_check=True)
                                else:
                                    for bi2 in range(len(blks)):
                                        nc.tensor.matmul(
                                            pSUM[:, pair_lo:TT],
                                            lhsT=ones_bf[:],
                                            rhs=expP[:, bi2, pair_lo:TT],
                                            start=(pi == 0 and bi2 == 0),
                                            stop=(pi == npair - 1
                                                  and bi2 == len(blks) - 1),
                                            skip_group_check=True)
                                for bi, b in enumerate(blks):
                                    s0 = b.sb * 128
                                    nc.tensor.matmul(
                                        pOT[:, b.lo:TT],
                                        lhsT=Vn[:, s0:s0 + 128],
                                        rhs=expP[:, bi, b.lo:TT],
                                        start=(pi == 0 and bi == 0),
                                        stop=(pi == npair - 1
                                              and bi == len(blks) - 1),
                                        skip_group_check=True)
                            recip = ats.tile([128, TT], F32, name="recip",
                                             bufs=2)
                            nc.vector.reciprocal_approx_fast(
                                out=recip[:], in_=pSUM[:])
                            tmpn = ats.tile([128, TT], DT, name="tmpn",
                                            bufs=4)
                            nc.vector.tensor_mul(tmpn[:], pOT[:], recip[:])
                            c0 = (tt % 2) * 4
                            nc.sync.dma_start(
                                out=attT_perm[tg][c0:c0 + 4,
                                                  h * HD:(h + 1) * HD, :]
                                .rearrange("c p w -> p c w"),
                                in_=tmpn[:].rearrange("p (c w) -> p c w",
                                                      c=4))

                    # fire this token group's all-to-all under the next
                    # group's compute, and fetch its slices back
                    nc.gpsimd.collective_compute(
                        "AllToAll", mybir.AluOpType.bypass,
                        replica_groups=[list(range(NCORES))],
                        ins=[attT_perm[tg][:].opt()],
                        outs=[a2a_out[tg][:].opt()])
                    a_g = p3e.tile([128, DBLK, 128], DT, name="attS",
                                   bufs=NTG)
                    nc.gpsimd.dma_start(
                        out=a_g[:],
                        in_=a2a_out[tg][:].rearrange("(jb p) w -> p jb w",
                                                     p=128))
                    attS[tg] = a_g

            # ---------------- output projection (g-outer) -----------------
            with tc.tile_pool(name="p3psum", bufs=1, space="PSUM") as p3p, \
                 tc.tile_pool(name="p3sbuf", bufs=1) as p3s:
                OG = 1024
                wo_res1 = p3s.tile([128, DBLK, DIM // 2], DT,
                                   name="wo_res1")
                for jlo, jhi in ((0, 4), (4, 8), (8, 12), (12, DBLK)):
                    nc.scalar.dma_start(
                        out=wo_res1[:, jlo:jhi, :],
                        in_=wo_r[:, jlo:jhi, DIM // 2:DIM])
                wo_half = [wo_res0, wo_res1]
                for g in range(NTG):
                    for og in range(DIM // OG):
                        po = p3p.tile([128, 2, TT], F32, name="po", bufs=2)
                        for jb in range(DBLK):
                            for ods in range(OG // TT):
                                nc.tensor.matmul(
                                    po[:, ods, :],
                                    lhsT=attS[g][:, jb, :],
                                    rhs=wo_half[og][:, jb,
                                                    ods * TT:
                                                    (ods + 1) * TT],
                                    start=(jb == 0), stop=(jb == DBLK - 1),
                                    skip_group_check=True)
                        ot = p3s.tile([128, OG], F32, name="ot", bufs=4)
                        nc.vector.tensor_copy(
                            ot[:, 0:OG // 2],
                            po[:].rearrange("p a b -> p (a b)")
                            [:, 0:OG // 2])
                        nc.scalar.activation(
                            ot[:, OG // 2:OG],
                            po[:].rearrange("p a b -> p (a b)")
                            [:, OG // 2:OG], COPY)
                        nc.sync.dma_start(
                            out=out_d[g * 128:(g + 1) * 128,
                                      og * OG:(og + 1) * OG],
                            in_=ot[:])

    nc.compile()
    return nc


def _prep_inputs(x, wq, wk, wv, wo, freqs_cos, freqs_sin):
    """Host-side transforms; returns the per-core in_maps."""
    perm = np.concatenate([np.arange(0, HD, 2), np.arange(1, HD, 2)])
    scale = 1.0 / np.sqrt(HD)
    wq_p = wq.reshape(DIM, HEADS, HD)[:, :, perm] * scale   # [DIM, 16, 128]
    wk_p = wk.reshape(DIM, KVH, HD)[:, :, perm]             # [DIM, 4, 128]
    wv_r = wv.reshape(DIM, KVH, HD)                         # [DIM, 4, 128]

    xT = np.ascontiguousarray(x.T).astype(BF16)
    wo_b = np.ascontiguousarray(wo).astype(BF16)

    cosT = np.ascontiguousarray(freqs_cos.T)                # [64, T]
    sinT = np.ascontiguousarray(freqs_sin.T)
    cos2 = np.concatenate([cosT, cosT], axis=0).astype(BF16)   # [128, T]
    sin2 = np.concatenate([-sinT, sinT], axis=0).astype(BF16)
    p64 = np.zeros((HD, HD), np.float32)
    p64[(np.arange(HD) + 64) % HD, np.arange(HD)] = 1.0
    p64 = p64.astype(BF16)
    # triangle bias (rhs=ident): bias[s, c] = lb[c, s] = NEGB iff s > c
    lb = np.where(np.arange(128)[None, :] > np.arange(128)[:, None],
                  NEGB, 0.0).astype(BF16)                   # lb[c, s]

    in_maps = []
    for c in range(NCORES):
        g = c // 2
        wqkv = np.concatenate(
            [wq_p[:, 2 * c], wq_p[:, 2 * c + 1], wk_p[:, g], wv_r[:, g]],
            axis=1).astype(BF16)                             # [DIM, 512]
        in_maps.append({
            "xT": xT, "wqkv": np.ascontiguousarray(wqkv), "wo": wo_b,
            "cos2": cos2, "sin2": sin2, "p64": p64, "lb": lb,
        })
    return in_maps


def kernel(x, wq, wk, wv, wo, freqs_cos, freqs_sin, seq_ids):
    x = np.asarray(x, np.float32)
    wq = np.asarray(wq, np.float32)
    wk = np.asarray(wk, np.float32)
    wv = np.asarray(wv, np.float32)
    wo = np.asarray(wo, np.float32)
    freqs_cos = np.asarray(freqs_cos, np.float32)
    freqs_sin = np.asarray(freqs_sin, np.float32)
    seq_ids = np.asarray(seq_ids)

    block_list, fast, masks_arr = _block_structure(seq_ids)
    if FORCE_MASKS:
        fast = False
    nc = _build_program(block_list, fast, masks_arr.shape[0])
    in_maps = _prep_inputs(x, wq, wk, wv, wo, freqs_cos, freqs_sin)
    for m in in_maps:
        m["masks"] = masks_arr

    trace = bool(os.environ.get("BASS_KERNEL_TRACE"))
    if trace:
        try:
            sys.path.insert(0, "/root/problem")
            import axon_shim
            axon_shim.install()
        except ImportError:
            pass
    res = None
    for attempt in range(3):
        try:
            res = run_bass_kernel_spmd(
                nc, in_maps, core_ids=list(range(NCORES)), trace=trace)
            break
        except Exception:
            if attempt == 2:
                raise
            import time as _time
            import jax as _jax
            _jax.clear_caches()
            _time.sleep(5)
    if trace:
        print(f"HW exec time: {res.exec_time_ns} ns")
        kernel.last_exec_time_ns = res.exec_time_ns
        kernel.last_results = res
    out = np.empty((T, DIM), np.float32)
    for c in range(NCORES):
        oc = res.results[c]["out"]
        for g in range(NTG):
            out[g * TG + c * 128:g * TG + (c + 1) * 128] = \
                oc[g * 128:(g + 1) * 128]
    return out
